# revision 5
# baseline (speedup 1.0000x reference)
"""Trainium2 Bass kernel: 7x7 local window attention (ConvNDAttention).

Input  X: [4, 64, 64, 256] fp32 (channel-last).
Output:   [4, 58, 58, 256] fp32.

For each output position (b, r, w): 7x7 input window rows r..r+6, cols
w..w+6; query = center cell (r+3, w+3); keys/values = the other 48 cells.
out = softmax(q . K / 16) @ K.

Sharding: 8 cores = 4 batches x 2 row-halves (30 output rows each, 2-row
overlap).  Per core, 18 tiles of 10x10 queries; each tile's keys are a 16x16
input patch (256 keys, 2 chunks of 128).

Host-side layouts per core (bf16):
  xp [128, 6, 2, 576]  channel-major "column panels": panel t covers input
      cols W0S[t]..W0S[t]+15, all 36 rows; free index s = row*16 + col.
      Chunk k = channels 128k..128k+127 on partitions.
  vv [128, 6, 3, 2, 257]  spatial-major V tiles for every (panel, r0, chunk)
      plus trailing ones column (row-sum trick).  Pre-baked on host so the
      whole thing loads with big contiguous DMA descriptors (the former
      per-tile strided loads were 4608 x 514B packets).
  mk [128, 2, 2, 100] 0/1 window-validity mask, duplicated for tile pairs.

Processing order: panel-pair-major (3 pair-columns x 3 row-origins) so input
DMA for pair p+1 prefetches under pair p's compute.  Two tiles share one
PSUM score bank -> exp/mask run once per pair over 400 elements, halving
fixed per-op engine overheads.  Output is staged in SBUF (bf16) and leaves
in 3 big contiguous DMAs.

Per-pair pipeline:
  1. S^T [128k, 2tt, 2j, 100q] = Xkey^T . Xq  (PE bf16, fp32 PSUM, 8 MMs)
  2. E = exp(S/16)  PSUM->SBUF bf16           (ACT, one op per pair)
  3. E *= mask                                 (DVE, one op per pair)
  4. AV [100, 257] = E.T @ V per tile (ones col -> col 256 = row sums) (PE)
  5. rinv = 1/AV[:,256] (DVE); out tile = AV[:,:256]*rinv -> obuf bf16
     (alternating ACT / DVE per tile to balance engine load)
"""

import numpy as np
import ml_dtypes

import concourse.bass as bass
import concourse.bacc as bacc
import concourse.mybir as mybir
import concourse.tile as tile

BF16 = ml_dtypes.bfloat16

# ---------------- geometry (hardcoded for X [4,64,64,256]) ----------------
B, H, W, C = 4, 64, 64, 256
HO, WO = H - 6, W - 6          # 58 x 58 output
N_CORES = 8
SH_ROWS_IN = 36                # input rows per shard
SH_ROWS_OUT = 30               # output rows per shard
R0S = [0, 10, 20]              # tile row origins (shard-local output rows)
W0S = [0, 10, 20, 30, 40, 48]  # tile col origins
NPAN = len(W0S)
NPAIR = NPAN // 2
QT = 10                        # query tile side
KT = 16                        # key patch side
NQ = QT * QT                   # 100 queries per tile
PAN = SH_ROWS_IN * KT          # 576 panel spatial positions


def _build_mask():
    """mk [128, 2tt, 2j, 100]: chunked-key x query validity (bf16 0/1)."""
    m = np.zeros((2, 128, NQ), dtype=np.float32)
    for j in range(2):
        for p in range(128):
            kh = 8 * j + p // KT
            kw = p % KT
            for q in range(NQ):
                qh, qw = q // QT, q % QT
                dy, dx = kh - qh, kw - qw
                if 0 <= dy <= 6 and 0 <= dx <= 6 and not (dy == 3 and dx == 3):
                    m[j, p, q] = 1.0
    mk1 = np.ascontiguousarray(m.transpose(1, 0, 2))          # [128, 2, 100]
    mk2 = np.broadcast_to(mk1[:, None], (128, 2, 2, NQ))
    return np.ascontiguousarray(mk2).astype(BF16)


_MASK = _build_mask()

_NC_CACHE = None


def _build_bass():
    global _NC_CACHE
    if _NC_CACHE is not None:
        return _NC_CACHE
    nc = bacc.Bacc("TRN2")
    dt = mybir.dt

    # one interleaved input tensor: per (partition, panel): 1152 elems of
    # channel-major panel + 1542 elems of V tiles -> ONE DMA per panel
    # (each dma_start costs ~0.6us of issue time on the queue engine)
    PSEG = 2 * PAN                 # 1152
    VSEG = 3 * 2 * (C + 1)         # 1542
    SEG = PSEG + VSEG              # 2694
    xpv = nc.dram_tensor("xpv", [128, NPAN, SEG], dt.bfloat16,
                         kind="ExternalInput")
    mk = nc.dram_tensor("mk", [128, 2, 2, NQ], dt.bfloat16,
                        kind="ExternalInput")
    out = nc.dram_tensor("out", [NPAIR, NQ, 3, 2, C], dt.bfloat16,
                         kind="ExternalOutput")

    with tile.TileContext(nc) as tc:
        with (
            tc.tile_pool(name="const", bufs=1) as const_pool,
            tc.tile_pool(name="ework", bufs=4) as e_pool,
            tc.tile_pool(name="rwork", bufs=4) as r_pool,
            tc.tile_pool(name="ps_s", bufs=3, space="PSUM") as ps_s,
            tc.tile_pool(name="ps_av", bufs=2, space="PSUM") as ps_av,
        ):
            xpv_all = const_pool.tile([128, NPAN, SEG], dt.bfloat16,
                                      tag="xpv")
            mk_sb = const_pool.tile([128, 2, 2, NQ], dt.bfloat16, tag="mk")
            obuf = const_pool.tile([NQ, NPAIR, 3, 2, C], dt.bfloat16,
                                   tag="obuf")

            nc.sync.dma_start(out=mk_sb[:, :, :, :], in_=mk[:, :, :, :])
            # per-panel loads in consumption order -> prefetch pipeline
            for t in range(NPAN):
                nc.sync.dma_start(out=xpv_all[:, t], in_=xpv[:, t])

            def panel(ti, k):
                return xpv_all[:, ti, k * PAN : (k + 1) * PAN]

            def vtile(ti, ri, j):
                off = PSEG + (ri * 2 + j) * (C + 1)
                return xpv_all[:, ti, off : off + C + 1]

            for pr in range(NPAIR):
                tis = (2 * pr, 2 * pr + 1)
                for ri, r0 in enumerate(R0S):
                    # ---- scores S^T for the tile pair (one PSUM bank) ----
                    st = ps_s.tile([128, 2, 2, NQ], dt.float32, tag="st")
                    for tt, ti in enumerate(tis):
                        for j in range(2):
                            for k in range(2):
                                pan = panel(ti, k)
                                keys = pan[:, (r0 + 8 * j) * KT :
                                            (r0 + 8 * j) * KT + 128]
                                qrys = pan.rearrange(
                                    "p (h w) -> p h w", w=KT
                                )[:, r0 + 3 : r0 + 3 + QT, 3 : 3 + QT]
                                nc.tensor.matmul(st[:, tt, j, :], lhsT=keys,
                                                 rhs=qrys, start=(k == 0),
                                                 stop=(k == 1))

                    # ---- E = exp(S/16) -> bf16 SBUF; mask (pair-batched) --
                    e = e_pool.tile([128, 2, 2, NQ], dt.bfloat16, tag="e")
                    nc.scalar.activation(e[:, :, :, :], st[:, :, :, :],
                                         mybir.ActivationFunctionType.Exp,
                                         scale=1.0 / 16.0)
                    nc.gpsimd.tensor_mul(e[:, :, :, :], e[:, :, :, :],
                                         mk_sb[:, :, :, :])

                    # ---- AV for both tiles into one 2-bank PSUM tile ----
                    av = ps_av.tile([NQ, 2, 512], dt.float32, tag="av")
                    for tt, ti in enumerate(tis):
                        for j in range(2):
                            nc.tensor.matmul(av[:, tt, 0 : C + 1],
                                             lhsT=e[:, tt, j, :],
                                             rhs=vtile(ti, ri, j),
                                             start=(j == 0), stop=(j == 1))
                    # one reciprocal for the pair (strided row-sum column)
                    rinv = r_pool.tile([NQ, 2], dt.float32, tag="rinv")
                    nc.vector.reciprocal(rinv[:, :], av[:, :, C : C + 1])
                    for tt, ti in enumerate(tis):
                        dst = obuf[:, pr, ri, tt, :]
                        if ri == 0 and tt == 0:
                            nc.scalar.mul(dst, av[:, tt, 0:C],
                                          rinv[:, tt : tt + 1])
                        else:
                            nc.vector.tensor_scalar_mul(dst, av[:, tt, 0:C],
                                                        rinv[:, tt : tt + 1])

                # ---- store this pair-column (one big contiguous DMA) ----
                nc.sync.dma_start(out=out[pr], in_=obuf[:, pr, :, :, :])

    nc.compile()
    _NC_CACHE = nc
    return nc


def _prep_inputs(X):
    X = np.ascontiguousarray(np.asarray(X, dtype=np.float32))
    in_maps = []
    mk = _MASK
    for c in range(N_CORES):
        b, half = c // 2, c % 2
        r_in0 = 0 if half == 0 else H - SH_ROWS_IN          # 0 or 28
        shard = X[b, r_in0 : r_in0 + SH_ROWS_IN]            # [36, 64, 256]
        shard_bf = shard.astype(BF16)
        # interleaved per-panel segments: [128, 6, 2*576 + 3*2*257]
        PSEG = 2 * PAN
        VSEG = 3 * 2 * (C + 1)
        xpv = np.empty((NPAN, 128, PSEG + VSEG), dtype=BF16)
        for t, w0 in enumerate(W0S):
            sl = shard_bf[:, w0 : w0 + KT, :]               # [36, 16, 256]
            slT = sl.reshape(PAN, C).T                      # [256, 576]
            xpv[t, :, :PSEG] = slT.reshape(2, 128, PAN).transpose(
                1, 0, 2).reshape(128, PSEG)
            vseg = np.empty((3, 2, 128, C + 1), dtype=BF16)
            for ri, r0 in enumerate(R0S):
                for j in range(2):
                    patch = shard_bf[r0 + 8 * j : r0 + 8 * j + 8,
                                     w0 : w0 + KT, :]       # [8, 16, 256]
                    vseg[ri, j, :, :C] = patch.reshape(128, C)
                    vseg[ri, j, :, C] = np.asarray(1.0, dtype=BF16)
            xpv[t, :, PSEG:] = vseg.transpose(2, 0, 1, 3).reshape(128, VSEG)
        xpvt = np.ascontiguousarray(xpv.transpose(1, 0, 2))
        in_maps.append({"xpv": xpvt, "mk": mk})
    return in_maps


def _gather_simple(results):
    full = np.empty((B, HO, WO, C), dtype=np.float32)
    ov = 2 * SH_ROWS_OUT - HO                               # overlap rows = 2
    for c in range(N_CORES):
        b, half = c // 2, c % 2
        o = np.asarray(results[c]["out"], dtype=np.float32)  # [3,100,3,2,256]
        loc = np.empty((SH_ROWS_OUT, WO, C), dtype=np.float32)
        for pr in range(NPAIR):
            for ri, r0 in enumerate(R0S):
                for tt in range(2):
                    w0 = W0S[2 * pr + tt]
                    blk = o[pr, :, ri, tt, :].reshape(QT, QT, C)
                    loc[r0 : r0 + QT, w0 : w0 + QT] = blk
        if half == 0:
            full[b, :SH_ROWS_OUT] = loc
        else:
            full[b, SH_ROWS_OUT:] = loc[ov:]
    return full


def _run(X, trace=False, **kw):
    from concourse.bass_utils import run_bass_kernel_spmd

    nc = _build_bass()
    in_maps = _prep_inputs(X)
    res = run_bass_kernel_spmd(nc, in_maps, core_ids=list(range(N_CORES)),
                               trace=trace, **kw)
    return res


def kernel(X):
    res = _run(X, trace=False)
    return _gather_simple(res.results)


# revision 6
# speedup vs baseline: 1.0649x; 1.0649x over previous
"""Trainium2 Bass kernel: 7x7 local window attention (ConvNDAttention).

Input  X: [4, 64, 64, 256] fp32 (channel-last).
Output:   [4, 58, 58, 256] fp32.

For each output position (b, r, w): 7x7 input window rows r..r+6, cols
w..w+6; query = center cell (r+3, w+3); keys/values = the other 48 cells.
out = softmax(q . K / 16) @ K.

Sharding: 8 cores = 4 batches x 2 row-halves (30 output rows each, 2-row
overlap).  Per core, 18 tiles of 10x10 queries; each tile's keys are a 16x16
input patch (256 keys, 2 chunks of 128).

Host-side layouts per core (bf16):
  xp [128, 6, 2, 576]  channel-major "column panels": panel t covers input
      cols W0S[t]..W0S[t]+15, all 36 rows; free index s = row*16 + col.
      Chunk k = channels 128k..128k+127 on partitions.
  vv [128, 6, 3, 2, 257]  spatial-major V tiles for every (panel, r0, chunk)
      plus trailing ones column (row-sum trick).  Pre-baked on host so the
      whole thing loads with big contiguous DMA descriptors (the former
      per-tile strided loads were 4608 x 514B packets).
  mk [128, 2, 2, 100] 0/1 window-validity mask, duplicated for tile pairs.

Processing order: panel-pair-major (3 pair-columns x 3 row-origins) so input
DMA for pair p+1 prefetches under pair p's compute.  Two tiles share one
PSUM score bank -> exp/mask run once per pair over 400 elements, halving
fixed per-op engine overheads.  Output is staged in SBUF (bf16) and leaves
in 3 big contiguous DMAs.

Per-pair pipeline:
  1. S^T [128k, 2tt, 2j, 100q] = Xkey^T . Xq  (PE bf16, fp32 PSUM, 8 MMs)
  2. E = exp(S/16)  PSUM->SBUF bf16           (ACT, one op per pair)
  3. E *= mask                                 (DVE, one op per pair)
  4. AV [100, 257] = E.T @ V per tile (ones col -> col 256 = row sums) (PE)
  5. rinv = 1/AV[:,256] (DVE); out tile = AV[:,:256]*rinv -> obuf bf16
     (alternating ACT / DVE per tile to balance engine load)
"""

import numpy as np
import ml_dtypes

import concourse.bass as bass
import concourse.bacc as bacc
import concourse.mybir as mybir
import concourse.tile as tile

BF16 = ml_dtypes.bfloat16

# ---------------- geometry (hardcoded for X [4,64,64,256]) ----------------
B, H, W, C = 4, 64, 64, 256
HO, WO = H - 6, W - 6          # 58 x 58 output
N_CORES = 8
SH_ROWS_IN = 36                # input rows per shard
SH_ROWS_OUT = 30               # output rows per shard
R0S = [0, 10, 20]              # tile row origins (shard-local output rows)
W0S = [0, 10, 20, 30, 40, 48]  # tile col origins
NPAN = len(W0S)
NPAIR = NPAN // 2
QT = 10                        # query tile side
KT = 16                        # key patch side
NQ = QT * QT                   # 100 queries per tile
PAN = SH_ROWS_IN * KT          # 576 panel spatial positions


def _build_mask():
    """mk [128, 2tt, 2j, 100]: chunked-key x query validity (bf16 0/1)."""
    m = np.zeros((2, 128, NQ), dtype=np.float32)
    for j in range(2):
        for p in range(128):
            kh = 8 * j + p // KT
            kw = p % KT
            for q in range(NQ):
                qh, qw = q // QT, q % QT
                dy, dx = kh - qh, kw - qw
                if 0 <= dy <= 6 and 0 <= dx <= 6 and not (dy == 3 and dx == 3):
                    m[j, p, q] = 1.0
    mk1 = np.ascontiguousarray(m.transpose(1, 0, 2))          # [128, 2, 100]
    mk2 = np.broadcast_to(mk1[:, None], (128, 2, 2, NQ))
    return np.ascontiguousarray(mk2).astype(BF16)


_MASK = _build_mask()

_NC_CACHE = None


def _build_bass():
    global _NC_CACHE
    if _NC_CACHE is not None:
        return _NC_CACHE
    nc = bacc.Bacc("TRN2")
    dt = mybir.dt

    # one interleaved input tensor: per (partition, panel): 1152 elems of
    # channel-major panel + 1542 elems of V tiles -> ONE DMA per panel
    # (each dma_start costs ~0.6us of issue time on the queue engine)
    PSEG = 2 * PAN                 # 1152
    VSEG = 3 * 2 * (C + 1)         # 1542
    SEG = PSEG + VSEG              # 2694
    xpv = nc.dram_tensor("xpv", [128, NPAN, SEG], dt.bfloat16,
                         kind="ExternalInput")
    mk = nc.dram_tensor("mk", [128, 2, 2, NQ], dt.bfloat16,
                        kind="ExternalInput")
    out = nc.dram_tensor("out", [NPAIR, NQ, 3, 2, C], dt.bfloat16,
                         kind="ExternalOutput")

    with tile.TileContext(nc) as tc:
        with (
            tc.tile_pool(name="const", bufs=1) as const_pool,
            tc.tile_pool(name="ework", bufs=4) as e_pool,
            tc.tile_pool(name="rwork", bufs=4) as r_pool,
            tc.tile_pool(name="ps_s", bufs=3, space="PSUM") as ps_s,
            tc.tile_pool(name="ps_av", bufs=2, space="PSUM") as ps_av,
        ):
            xpv_all = const_pool.tile([128, NPAN, SEG], dt.bfloat16,
                                      tag="xpv")
            mk_sb = const_pool.tile([128, 2, 2, NQ], dt.bfloat16, tag="mk")
            obuf = const_pool.tile([NQ, NPAIR, 3, 2, C], dt.bfloat16,
                                   tag="obuf")

            nc.scalar.dma_start(out=mk_sb[:, :, :, :], in_=mk[:, :, :, :])
            # per-panel loads in consumption order -> prefetch pipeline;
            # alternate HWDGE issue engines (each dma_start costs ~0.65us
            # of issue time on its queue engine)
            for t in range(NPAN):
                eng = nc.sync if t % 2 == 0 else nc.scalar
                eng.dma_start(out=xpv_all[:, t], in_=xpv[:, t])

            def panel(ti, k):
                return xpv_all[:, ti, k * PAN : (k + 1) * PAN]

            def vtile(ti, ri, j):
                off = PSEG + (ri * 2 + j) * (C + 1)
                return xpv_all[:, ti, off : off + C + 1]

            for pr in range(NPAIR):
                tis = (2 * pr, 2 * pr + 1)
                for ri, r0 in enumerate(R0S):
                    # ---- scores S^T for the tile pair (one PSUM bank) ----
                    st = ps_s.tile([128, 2, 2, NQ], dt.float32, tag="st")
                    for tt, ti in enumerate(tis):
                        for j in range(2):
                            for k in range(2):
                                pan = panel(ti, k)
                                keys = pan[:, (r0 + 8 * j) * KT :
                                            (r0 + 8 * j) * KT + 128]
                                qrys = pan.rearrange(
                                    "p (h w) -> p h w", w=KT
                                )[:, r0 + 3 : r0 + 3 + QT, 3 : 3 + QT]
                                nc.tensor.matmul(st[:, tt, j, :], lhsT=keys,
                                                 rhs=qrys, start=(k == 0),
                                                 stop=(k == 1))

                    # ---- E = exp(S/16) -> bf16 SBUF; mask (pair-batched) --
                    e = e_pool.tile([128, 2, 2, NQ], dt.bfloat16, tag="e")
                    nc.scalar.activation(e[:, :, :, :], st[:, :, :, :],
                                         mybir.ActivationFunctionType.Exp,
                                         scale=1.0 / 16.0)
                    nc.gpsimd.tensor_mul(e[:, :, :, :], e[:, :, :, :],
                                         mk_sb[:, :, :, :])

                    # ---- AV for both tiles into one 2-bank PSUM tile ----
                    av = ps_av.tile([NQ, 2, 512], dt.float32, tag="av")
                    for tt, ti in enumerate(tis):
                        for j in range(2):
                            nc.tensor.matmul(av[:, tt, 0 : C + 1],
                                             lhsT=e[:, tt, j, :],
                                             rhs=vtile(ti, ri, j),
                                             start=(j == 0), stop=(j == 1))
                    # one reciprocal for the pair (strided row-sum column)
                    rinv = r_pool.tile([NQ, 2], dt.float32, tag="rinv")
                    nc.vector.reciprocal(rinv[:, :], av[:, :, C : C + 1])
                    for tt, ti in enumerate(tis):
                        dst = obuf[:, pr, ri, tt, :]
                        if ri == 0 and tt == 0:
                            nc.scalar.mul(dst, av[:, tt, 0:C],
                                          rinv[:, tt : tt + 1])
                        else:
                            nc.vector.tensor_scalar_mul(dst, av[:, tt, 0:C],
                                                        rinv[:, tt : tt + 1])

                # ---- store this pair-column (one big contiguous DMA) ----
                nc.sync.dma_start(out=out[pr], in_=obuf[:, pr, :, :, :])

    nc.compile()
    _NC_CACHE = nc
    return nc


def _prep_inputs(X):
    X = np.ascontiguousarray(np.asarray(X, dtype=np.float32))
    in_maps = []
    mk = _MASK
    for c in range(N_CORES):
        b, half = c // 2, c % 2
        r_in0 = 0 if half == 0 else H - SH_ROWS_IN          # 0 or 28
        shard = X[b, r_in0 : r_in0 + SH_ROWS_IN]            # [36, 64, 256]
        shard_bf = shard.astype(BF16)
        # interleaved per-panel segments: [128, 6, 2*576 + 3*2*257]
        PSEG = 2 * PAN
        VSEG = 3 * 2 * (C + 1)
        xpv = np.empty((NPAN, 128, PSEG + VSEG), dtype=BF16)
        for t, w0 in enumerate(W0S):
            sl = shard_bf[:, w0 : w0 + KT, :]               # [36, 16, 256]
            slT = sl.reshape(PAN, C).T                      # [256, 576]
            xpv[t, :, :PSEG] = slT.reshape(2, 128, PAN).transpose(
                1, 0, 2).reshape(128, PSEG)
            vseg = np.empty((3, 2, 128, C + 1), dtype=BF16)
            for ri, r0 in enumerate(R0S):
                for j in range(2):
                    patch = shard_bf[r0 + 8 * j : r0 + 8 * j + 8,
                                     w0 : w0 + KT, :]       # [8, 16, 256]
                    vseg[ri, j, :, :C] = patch.reshape(128, C)
                    vseg[ri, j, :, C] = np.asarray(1.0, dtype=BF16)
            xpv[t, :, PSEG:] = vseg.transpose(2, 0, 1, 3).reshape(128, VSEG)
        xpvt = np.ascontiguousarray(xpv.transpose(1, 0, 2))
        in_maps.append({"xpv": xpvt, "mk": mk})
    return in_maps


def _gather_simple(results):
    full = np.empty((B, HO, WO, C), dtype=np.float32)
    ov = 2 * SH_ROWS_OUT - HO                               # overlap rows = 2
    for c in range(N_CORES):
        b, half = c // 2, c % 2
        o = np.asarray(results[c]["out"], dtype=np.float32)  # [3,100,3,2,256]
        loc = np.empty((SH_ROWS_OUT, WO, C), dtype=np.float32)
        for pr in range(NPAIR):
            for ri, r0 in enumerate(R0S):
                for tt in range(2):
                    w0 = W0S[2 * pr + tt]
                    blk = o[pr, :, ri, tt, :].reshape(QT, QT, C)
                    loc[r0 : r0 + QT, w0 : w0 + QT] = blk
        if half == 0:
            full[b, :SH_ROWS_OUT] = loc
        else:
            full[b, SH_ROWS_OUT:] = loc[ov:]
    return full


def _run(X, trace=False, **kw):
    from concourse.bass_utils import run_bass_kernel_spmd

    nc = _build_bass()
    in_maps = _prep_inputs(X)
    res = run_bass_kernel_spmd(nc, in_maps, core_ids=list(range(N_CORES)),
                               trace=trace, **kw)
    return res


def kernel(X):
    res = _run(X, trace=False)
    return _gather_simple(res.results)
